# revision 8
# baseline (speedup 1.0000x reference)
"""Trainium2 Bass kernel for the per-cell-MLP "MAR one-sided missingness" model.

Model (per cell (n,t) of a 1024x128 grid):
    xc     = X[n, col_idx[n,t]]
    h      = relu(W_in[n,t,:,0]*xc + W_in[n,t,:,1]*X[n,t] + b_in[n,t,:])   # [H]
    out    = sigmoid(dot(W_out[n,t,:], h) + b_out[n,t])

Sharding: rows N split across 8 cores (128 rows each), fully data parallel.

Per-core layout: partition dim = t (128 cells of one row), free dim = h.
The neighbor gather X[n, col_idx[n,t]] runs on the PE as a one-hot matmul:
one-hot masks (a host-staged re-encoding of col_idx) are f16 stationaries;
X rides as an f16 hi/lo split (lo pre-scaled by 2^12 to avoid denormals) so
the gathered values are exact to ~2.5e-7 relative.

Per superblock of G=16 rows:
  DMA  : wt  = wall[t, n, (w0|w1|b), h] slice   (3 MB weight stream)
  DMA  : wo  = woall[t, n, h] slice             (1 MB, contiguous W_out)
  DMA  : oh  = one-hot f16 slice [j, (g t)]     (512 KB)
  PE   : xc2[:, 2g:2g+2] = oh_g^T @ [Xhi | Xlo*2^12][:, n]   (per row)
  DVE  : xc  = (xc2_lo * 2^-12) + xc2_hi        (batched, [128, G])
  ACT  : a0_g = w0_g * xc_g                     (per row, per-partition scale)
  DVE  : v_g  = (w1_g * x_g) + b_g              (per row, fused STT)
  Pool : u    = a0 + v                          (batched)
  DVE  : r    = (u max 0) * wo                  (batched STT, contiguous)
  DVE  : red[:, g] = sum_h r                    (batched reduce)
Epilogue: out = sigmoid(red + b_out^T), DMA out, host transposes back.

HBM-bandwidth bound: streams ~36 MB per core (~107 us at 340 GB/s).
"""

import ml_dtypes
import numpy as np

N, T, H = 1024, 128, 128
M = 8            # cores
NR = N // M      # rows per core
G = 16           # rows per superblock (one weight DMA)
S = NR // G
LO_SCALE = float(2 ** 12)

_cache = {}


def _build():
    if "nc" in _cache:
        return _cache["nc"]
    import concourse.bacc as bacc
    import concourse.mybir as mybir
    import concourse.tile as tile

    f32 = mybir.dt.float32
    f16 = mybir.dt.float16
    f8 = mybir.dt.float8e4
    Alu = mybir.AluOpType
    Act = mybir.ActivationFunctionType

    nc = bacc.Bacc()
    wall = nc.declare_dram_parameter("wall", [T, NR, 3, H], f32, isOutput=False)
    woall = nc.declare_dram_parameter("woall", [T, NR, H], f32, isOutput=False)
    ohall = nc.declare_dram_parameter("ohall", [128, NR * T], f8, isOutput=False)
    xt = nc.declare_dram_parameter("xt", [T, NR], f32, isOutput=False)
    xhl = nc.declare_dram_parameter("xhl", [128, NR, 2], f16, isOutput=False)
    bout = nc.declare_dram_parameter("bout", [T, NR], f32, isOutput=False)
    out = nc.declare_dram_parameter("out", [T, NR], f32, isOutput=True)

    with tile.TileContext(nc) as tc:
        with (
            tc.tile_pool(name="const", bufs=1) as constp,
            tc.tile_pool(name="wpool", bufs=3) as wpool,
            tc.tile_pool(name="wopool", bufs=3) as wopool,
            tc.tile_pool(name="ohp", bufs=3) as ohp,
            tc.tile_pool(name="work", bufs=2) as workp,
            tc.tile_pool(name="acc", bufs=1) as accp,
            tc.tile_pool(name="psxc", bufs=2, space="PSUM") as psxcp,
        ):
            xt_sb = constp.tile([T, NR], f32)
            nc.scalar.dma_start(xt_sb[:], xt[:])
            xhl_sb = constp.tile([128, NR * 2], f16)
            nc.scalar.dma_start(xhl_sb[:], xhl[:])
            bo_sb = constp.tile([T, NR], f32)
            nc.scalar.dma_start(bo_sb[:], bout[:])

            red = accp.tile([T, NR], f32)

            for s in range(S):
                wt = wpool.tile([128, G * 3 * H], f32, tag="wt")
                nc.sync.dma_start(wt[:], wall[:, s * G : (s + 1) * G])
                wo = wopool.tile([128, G * H], f32, tag="wo")
                nc.scalar.dma_start(wo[:], woall[:, s * G : (s + 1) * G])
                oh = ohp.tile([128, G * T], f8, tag="oh")
                nc.scalar.dma_start(
                    oh[:], ohall[:, s * G * T : (s + 1) * G * T]
                )

                xc2_ps = psxcp.tile([128, 2 * G], f32, tag="xc")
                for g in range(G):
                    n = s * G + g
                    nc.tensor.matmul(
                        xc2_ps[:, 2 * g : 2 * g + 2],
                        oh[:, g * T : (g + 1) * T],
                        xhl_sb[:, 2 * n : 2 * n + 2],
                        start=True,
                        stop=True,
                    )
                xc2_sb = workp.tile([128, 2 * G], f32, tag="xc2sb")
                nc.scalar.copy(xc2_sb[:], xc2_ps[:])
                xc_sb = workp.tile([128, G], f32, tag="xcsb")
                nc.vector.scalar_tensor_tensor(
                    xc_sb[:],
                    xc2_sb[:].rearrange("p (g k) -> p g k", k=2)[:, :, 1],
                    1.0 / LO_SCALE,
                    xc2_sb[:].rearrange("p (g k) -> p g k", k=2)[:, :, 0],
                    Alu.mult,
                    Alu.add,
                )

                a0 = workp.tile([128, G * H], f32, tag="a0")
                v = workp.tile([128, G * H], f32, tag="v")
                u = workp.tile([128, G * H], f32, tag="u")
                r = workp.tile([128, G * H], f32, tag="r")
                for g in range(G):
                    n = s * G + g
                    w0 = wt[:, (3 * g + 0) * H : (3 * g + 1) * H]
                    w1 = wt[:, (3 * g + 1) * H : (3 * g + 2) * H]
                    bb = wt[:, (3 * g + 2) * H : (3 * g + 3) * H]
                    nc.scalar.activation(
                        a0[:, g * H : (g + 1) * H],
                        w0,
                        Act.Copy,
                        scale=xc_sb[:, g : g + 1],
                    )
                    nc.vector.scalar_tensor_tensor(
                        v[:, g * H : (g + 1) * H],
                        w1,
                        xt_sb[:, n : n + 1],
                        bb,
                        Alu.mult,
                        Alu.add,
                    )
                HB = G * H // 2
                for hbl in range(2):
                    sl_ = slice(hbl * HB, (hbl + 1) * HB)
                    nc.gpsimd.tensor_tensor(u[:, sl_], a0[:, sl_], v[:, sl_], Alu.add)
                    nc.vector.scalar_tensor_tensor(
                        r[:, sl_], u[:, sl_], 0.0, wo[:, sl_], Alu.max, Alu.mult
                    )
                    nc.vector.tensor_reduce(
                        red[:, s * G + hbl * G // 2 : s * G + (hbl + 1) * G // 2],
                        r[:, sl_].rearrange("p (g h) -> p g h", g=G // 2),
                        axis=mybir.AxisListType.X,
                        op=Alu.add,
                    )

                r = workp.tile([128, G * H], f32, tag="r")
                nc.vector.scalar_tensor_tensor(
                    r[:], u[:], 0.0, wo[:], Alu.max, Alu.mult
                )
                nc.vector.tensor_reduce(
                    red[:, s * G : (s + 1) * G],
                    r[:].rearrange("p (g h) -> p g h", g=G),
                    axis=mybir.AxisListType.X,
                    op=Alu.add,
                )

            lg = workp.tile([T, NR], f32, tag="lg")
            nc.vector.tensor_tensor(lg[:], red[:], bo_sb[:], Alu.add)
            ot = workp.tile([T, NR], f32, tag="ot")
            nc.scalar.activation(ot[:], lg[:], Act.Sigmoid)
            nc.sync.dma_start(out[:], ot[:])

    nc.compile()
    _cache["nc"] = nc
    return nc


def make_in_maps(X, W_in, b_in, W_out, b_out, col_idx):
    X = np.asarray(X, dtype=np.float32)
    W_in = np.asarray(W_in, dtype=np.float32)
    b_in = np.asarray(b_in, dtype=np.float32)
    W_out = np.asarray(W_out, dtype=np.float32)
    b_out = np.asarray(b_out, dtype=np.float32)
    col_idx = np.asarray(col_idx)

    jj = np.arange(128)
    in_maps = []
    for c in range(M):
        sl = slice(c * NR, (c + 1) * NR)
        Wc = W_in[sl]  # [NR, T, H, 2]
        wall = np.empty((T, NR, 3, H), dtype=np.float32)
        wall[:, :, 0, :] = Wc[:, :, :, 0].transpose(1, 0, 2)
        wall[:, :, 1, :] = Wc[:, :, :, 1].transpose(1, 0, 2)
        wall[:, :, 2, :] = b_in[sl].transpose(1, 0, 2)
        woall = np.ascontiguousarray(W_out[sl].transpose(1, 0, 2))

        ohall = (col_idx[sl].reshape(1, -1) == jj[:, None]).astype(ml_dtypes.float8_e4m3)

        xtc = np.ascontiguousarray(X[sl].T)  # [t, n] f32
        xhi = xtc.astype(np.float16)
        xlo = ((xtc - xhi.astype(np.float32)) * LO_SCALE).astype(np.float16)
        xhl = np.stack([xhi, xlo], axis=-1)  # [128, NR, 2]

        in_maps.append(
            {
                "wall": wall,
                "woall": woall,
                "ohall": ohall,
                "xt": xtc,
                "xhl": xhl,
                "bout": np.ascontiguousarray(b_out[sl].T),
            }
        )
    return in_maps


def kernel(X, W_in, b_in, W_out, b_out, col_idx):
    from concourse.bass_utils import run_bass_kernel_spmd

    nc = _build()
    in_maps = make_in_maps(X, W_in, b_in, W_out, b_out, col_idx)
    res = run_bass_kernel_spmd(nc, in_maps, list(range(M))).results
    out = np.empty((N, T), np.float32)
    for c in range(M):
        out[c * NR : (c + 1) * NR] = res[c]["out"].T
    return out


# revision 9
# speedup vs baseline: 1.0131x; 1.0131x over previous
"""Trainium2 Bass kernel for the per-cell-MLP "MAR one-sided missingness" model.

Model (per cell (n,t) of a 1024x128 grid):
    xc     = X[n, col_idx[n,t]]
    h      = relu(W_in[n,t,:,0]*xc + W_in[n,t,:,1]*X[n,t] + b_in[n,t,:])   # [H]
    out    = sigmoid(dot(W_out[n,t,:], h) + b_out[n,t])

Sharding: rows N split across 8 cores (128 rows each), fully data parallel.

Per-core layout: partition dim = t (128 cells of one row), free dim = h.
The neighbor gather X[n, col_idx[n,t]] runs on the PE as a one-hot matmul:
one-hot masks (a host-staged re-encoding of col_idx) are f16 stationaries;
X rides as an f16 hi/lo split (lo pre-scaled by 2^12 to avoid denormals) so
the gathered values are exact to ~2.5e-7 relative.

Per superblock of G=16 rows:
  DMA  : wt  = wall[t, n, (w0|w1|b), h] slice   (3 MB weight stream)
  DMA  : wo  = woall[t, n, h] slice             (1 MB, contiguous W_out)
  DMA  : oh  = one-hot f16 slice [j, (g t)]     (512 KB)
  PE   : xc2[:, 2g:2g+2] = oh_g^T @ [Xhi | Xlo*2^12][:, n]   (per row)
  DVE  : xc  = (xc2_lo * 2^-12) + xc2_hi        (batched, [128, G])
  ACT  : a0_g = w0_g * xc_g                     (per row, per-partition scale)
  DVE  : v_g  = (w1_g * x_g) + b_g              (per row, fused STT)
  Pool : u    = a0 + v                          (batched)
  DVE  : r    = (u max 0) * wo                  (batched STT, contiguous)
  DVE  : red[:, g] = sum_h r                    (batched reduce)
Epilogue: out = sigmoid(red + b_out^T), DMA out, host transposes back.

HBM-bandwidth bound: streams ~36 MB per core (~107 us at 340 GB/s).
"""

import ml_dtypes
import numpy as np

N, T, H = 1024, 128, 128
M = 8            # cores
NR = N // M      # rows per core
G = 16           # rows per superblock (one weight DMA)
S = NR // G
LO_SCALE = float(2 ** 12)

_cache = {}


def _build():
    if "nc" in _cache:
        return _cache["nc"]
    import concourse.bacc as bacc
    import concourse.mybir as mybir
    import concourse.tile as tile

    f32 = mybir.dt.float32
    f16 = mybir.dt.float16
    f8 = mybir.dt.float8e4
    Alu = mybir.AluOpType
    Act = mybir.ActivationFunctionType

    nc = bacc.Bacc()
    wall = nc.declare_dram_parameter("wall", [T, NR, 3, H], f32, isOutput=False)
    woall = nc.declare_dram_parameter("woall", [T, NR, H], f32, isOutput=False)
    ohall = nc.declare_dram_parameter("ohall", [128, NR * T], f8, isOutput=False)
    xt = nc.declare_dram_parameter("xt", [T, NR], f32, isOutput=False)
    xhl = nc.declare_dram_parameter("xhl", [128, NR, 2], f16, isOutput=False)
    bout = nc.declare_dram_parameter("bout", [T, NR], f32, isOutput=False)
    out = nc.declare_dram_parameter("out", [T, NR], f32, isOutput=True)

    with tile.TileContext(nc) as tc:
        with (
            tc.tile_pool(name="const", bufs=1) as constp,
            tc.tile_pool(name="wpool", bufs=3) as wpool,
            tc.tile_pool(name="wopool", bufs=3) as wopool,
            tc.tile_pool(name="ohp", bufs=3) as ohp,
            tc.tile_pool(name="work", bufs=2) as workp,
            tc.tile_pool(name="acc", bufs=1) as accp,
            tc.tile_pool(name="psxc", bufs=2, space="PSUM") as psxcp,
        ):
            xt_sb = constp.tile([T, NR], f32)
            nc.scalar.dma_start(xt_sb[:], xt[:])
            xhl_sb = constp.tile([128, NR * 2], f16)
            nc.scalar.dma_start(xhl_sb[:], xhl[:])
            bo_sb = constp.tile([T, NR], f32)
            nc.scalar.dma_start(bo_sb[:], bout[:])

            red = accp.tile([T, NR], f32)

            for s in range(S):
                wt = wpool.tile([128, G * 3 * H], f32, tag="wt")
                nc.sync.dma_start(wt[:], wall[:, s * G : (s + 1) * G])
                wo = wopool.tile([128, G * H], f32, tag="wo")
                nc.scalar.dma_start(wo[:], woall[:, s * G : (s + 1) * G])
                oh = ohp.tile([128, G * T], f8, tag="oh")
                nc.scalar.dma_start(
                    oh[:], ohall[:, s * G * T : (s + 1) * G * T]
                )

                xc2_ps = psxcp.tile([128, 2 * G], f32, tag="xc")
                for g in range(G):
                    n = s * G + g
                    nc.tensor.matmul(
                        xc2_ps[:, 2 * g : 2 * g + 2],
                        oh[:, g * T : (g + 1) * T],
                        xhl_sb[:, 2 * n : 2 * n + 2],
                        start=True,
                        stop=True,
                    )
                xc2_sb = workp.tile([128, 2 * G], f32, tag="xc2sb")
                nc.scalar.copy(xc2_sb[:], xc2_ps[:])
                xc_sb = workp.tile([128, G], f32, tag="xcsb")
                nc.vector.scalar_tensor_tensor(
                    xc_sb[:],
                    xc2_sb[:].rearrange("p (g k) -> p g k", k=2)[:, :, 1],
                    1.0 / LO_SCALE,
                    xc2_sb[:].rearrange("p (g k) -> p g k", k=2)[:, :, 0],
                    Alu.mult,
                    Alu.add,
                )

                a0 = workp.tile([128, G * H], f32, tag="a0")
                v = workp.tile([128, G * H], f32, tag="v")
                u = workp.tile([128, G * H], f32, tag="u")
                r = workp.tile([128, G * H], f32, tag="r")
                for g in range(G):
                    n = s * G + g
                    w0 = wt[:, (3 * g + 0) * H : (3 * g + 1) * H]
                    w1 = wt[:, (3 * g + 1) * H : (3 * g + 2) * H]
                    bb = wt[:, (3 * g + 2) * H : (3 * g + 3) * H]
                    nc.scalar.activation(
                        a0[:, g * H : (g + 1) * H],
                        w0,
                        Act.Copy,
                        scale=xc_sb[:, g : g + 1],
                    )
                    nc.vector.scalar_tensor_tensor(
                        v[:, g * H : (g + 1) * H],
                        w1,
                        xt_sb[:, n : n + 1],
                        bb,
                        Alu.mult,
                        Alu.add,
                    )
                nc.gpsimd.tensor_tensor(u[:], a0[:], v[:], Alu.add)
                nc.vector.scalar_tensor_tensor(
                    r[:], u[:], 0.0, wo[:], Alu.max, Alu.mult
                )
                nc.vector.tensor_reduce(
                    red[:, s * G : (s + 1) * G],
                    r[:].rearrange("p (g h) -> p g h", g=G),
                    axis=mybir.AxisListType.X,
                    op=Alu.add,
                )

                r = workp.tile([128, G * H], f32, tag="r")
                nc.vector.scalar_tensor_tensor(
                    r[:], u[:], 0.0, wo[:], Alu.max, Alu.mult
                )
                nc.vector.tensor_reduce(
                    red[:, s * G : (s + 1) * G],
                    r[:].rearrange("p (g h) -> p g h", g=G),
                    axis=mybir.AxisListType.X,
                    op=Alu.add,
                )

            lg = workp.tile([T, NR], f32, tag="lg")
            nc.vector.tensor_tensor(lg[:], red[:], bo_sb[:], Alu.add)
            ot = workp.tile([T, NR], f32, tag="ot")
            nc.scalar.activation(ot[:], lg[:], Act.Sigmoid)
            nc.sync.dma_start(out[:], ot[:])

    nc.compile()
    _cache["nc"] = nc
    return nc


def make_in_maps(X, W_in, b_in, W_out, b_out, col_idx):
    X = np.asarray(X, dtype=np.float32)
    W_in = np.asarray(W_in, dtype=np.float32)
    b_in = np.asarray(b_in, dtype=np.float32)
    W_out = np.asarray(W_out, dtype=np.float32)
    b_out = np.asarray(b_out, dtype=np.float32)
    col_idx = np.asarray(col_idx)

    jj = np.arange(128)
    in_maps = []
    for c in range(M):
        sl = slice(c * NR, (c + 1) * NR)
        Wc = W_in[sl]  # [NR, T, H, 2]
        wall = np.empty((T, NR, 3, H), dtype=np.float32)
        wall[:, :, 0, :] = Wc[:, :, :, 0].transpose(1, 0, 2)
        wall[:, :, 1, :] = Wc[:, :, :, 1].transpose(1, 0, 2)
        wall[:, :, 2, :] = b_in[sl].transpose(1, 0, 2)
        woall = np.ascontiguousarray(W_out[sl].transpose(1, 0, 2))

        ohall = (col_idx[sl].reshape(1, -1) == jj[:, None]).astype(ml_dtypes.float8_e4m3)

        xtc = np.ascontiguousarray(X[sl].T)  # [t, n] f32
        xhi = xtc.astype(np.float16)
        xlo = ((xtc - xhi.astype(np.float32)) * LO_SCALE).astype(np.float16)
        xhl = np.stack([xhi, xlo], axis=-1)  # [128, NR, 2]

        in_maps.append(
            {
                "wall": wall,
                "woall": woall,
                "ohall": ohall,
                "xt": xtc,
                "xhl": xhl,
                "bout": np.ascontiguousarray(b_out[sl].T),
            }
        )
    return in_maps


def kernel(X, W_in, b_in, W_out, b_out, col_idx):
    from concourse.bass_utils import run_bass_kernel_spmd

    nc = _build()
    in_maps = make_in_maps(X, W_in, b_in, W_out, b_out, col_idx)
    res = run_bass_kernel_spmd(nc, in_maps, list(range(M))).results
    out = np.empty((N, T), np.float32)
    for c in range(M):
        out[c * NR : (c + 1) * NR] = res[c]["out"].T
    return out
